# revision 1
# baseline (speedup 1.0000x reference)
"""Trainium2 Bass kernel for nn_MimicLoss (masked-MSE mimic loss), v5.

Data-parallel over batch: 8 NeuronCores x 4 samples. Per core:
  1. rasterize per-sample union-of-positive-boxes masks: priors land with
     prior p = lane*64 + c on partition `lane`, chunk `c`. Coverage is
     built in 6 batched ops over [128, 64*64] (bf16 iota vs f32 bounds
     broadcast along the pixel axis) split across GpSimd and DVE, then
     one bf16 matmul per 128-prior chunk accumulates
       hit[h, (b,w)] += sum_p cov_y[p,h] * (cov_x[p,w] * pos[p,b])
     with the (cov_x * pos) products batched 8 chunks per DVE op.
     mask = hit > 0.
  2. stream s and t in [128, 4096] f32 tiles (2 MB DMAs, one sync-queue,
     interleaved s,t per unit; final unit split into 1 MB halves);
     d = s - t on DVE in [128, 2048] halves, interleaved with the raster
     work in DVE program order so the stream never backs up;
     d2 = Square(d) -> f32r on ACT per half (2x mode); channel-column
     sums via PE matmuls with a ones stationary into a [1, 2048] PSUM
     tile (4 bank-segments per round), one 8 KB copy out per round.
  3. each [1, 4096] colsum row is reshaped SBUF->SBUF (SWDGE queue, which
     carries no stream traffic) into one [64, (b, cc), 64] tile = the
     mask's own [h, w] layout; two fused DVE mult+reduce pairs produce
     all half-contributions. Final partition reduce via a ones matmul.
Host: gather [1, 12] per core (8 half-contributions + 4 mask sums), apply
divide by (msum * C) and the sequential empty-mask-resets-loss scan, /B.

Self-contained: shapes hardcoded for map_t2/map_s2 [32,256,64,64] f32,
priors [8192,4] f32, mimic_label [32,8192] int32.
"""
import sys

sys.path.insert(0, "/opt/trn_rl_repo")

import numpy as np

import concourse.bacc as bacc
import concourse.bass as bass
import concourse.tile as tile
from concourse import mybir
from concourse.alu_op_type import AluOpType as Op

F32 = mybir.dt.float32
F32R = mybir.dt.float32r
I32 = mybir.dt.int32
BF16 = mybir.dt.bfloat16
AF = mybir.ActivationFunctionType

B, C, H, W = 32, 256, 64, 64
P = 8192
N_CORES = 8
BPC = B // N_CORES          # samples per core
HW = H * W                  # 4096
HWH = HW // 2               # 2048
NCHUNK = 64                 # prior chunks; prior p = lane*64 + c
CC = C // 128               # channel chunks
NU = BPC * CC               # stream units (one per (sample, cc))
GCH = 8                     # chunks per xb4 batch group


def build_nc():
    nc = bacc.Bacc("TRN2", debug=False)

    s = nc.dram_tensor("s", [BPC, C, H, W], F32, kind="ExternalInput")
    t = nc.dram_tensor("t", [BPC, C, H, W], F32, kind="ExternalInput")
    priors = nc.dram_tensor("priors", [P, 4], F32, kind="ExternalInput")
    # labels_r[l, c*BPC + b] = mimic_label[b, l*64 + c] (host-permuted)
    labels_r = nc.dram_tensor("labels_r", [128, NCHUNK * BPC], I32, kind="ExternalInput")
    out = nc.dram_tensor("out", [1, 3 * BPC], F32, kind="ExternalOutput")

    s_flat = s[:].rearrange("b c h w -> b c (h w)")
    t_flat = t[:].rearrange("b c h w -> b c (h w)")
    # prior p = lane*64 + c -> priors_sb[lane, c, j] (contiguous load)
    priors_ap = priors[:].rearrange("(l c) j -> l c j", l=128)

    with tile.TileContext(nc) as tc:
        with (
            tc.tile_pool(name="const", bufs=1) as constp,
            tc.tile_pool(name="small", bufs=1) as small,
            tc.tile_pool(name="rast", bufs=1) as rastp,
            tc.tile_pool(name="xb4p", bufs=2) as xb4p,
            tc.tile_pool(name="stream_s", bufs=3) as pool_s,
            tc.tile_pool(name="stream_t", bufs=2) as pool_t,
            tc.tile_pool(name="d2hp", bufs=2) as d2hp,
            tc.tile_pool(name="csp", bufs=2) as csp,
            tc.tile_pool(name="ps_hit", bufs=1, space="PSUM") as ps_hit,
            tc.tile_pool(name="ps_cs", bufs=1, space="PSUM") as ps_cs,
            tc.tile_pool(name="ps_out", bufs=1, space="PSUM") as ps_out,
        ):
            # ---- constants & small inputs ----
            ones_f = constp.tile([128, 1], F32)
            nc.vector.memset(ones_f[:], 1.0)
            ones_bf = constp.tile([128, 1], BF16)
            nc.vector.memset(ones_bf[:], 1.0)
            ones_r = constp.tile([128, 1], F32R)
            nc.vector.tensor_copy(ones_r[:], ones_f[:])

            priors_sb = small.tile([128, NCHUNK, 4], F32)
            nc.scalar.dma_start(priors_sb[:], priors_ap)
            labels_sb = small.tile([128, NCHUNK, BPC], I32)
            nc.scalar.dma_start(
                labels_sb[:], labels_r[:].rearrange("l (c b) -> l c b", b=BPC)
            )

            # pos = (label > 0) as bf16 (flat 2D APs for fast copy modes)
            pos_f = small.tile([128, NCHUNK * BPC], F32)
            nc.vector.tensor_copy(
                pos_f[:], labels_sb[:].rearrange("p c b -> p (c b)")
            )
            nc.vector.tensor_single_scalar(pos_f[:], pos_f[:], 0.0, Op.is_gt)
            pos_bf = small.tile([128, NCHUNK, BPC], BF16)
            nc.vector.tensor_copy(
                pos_bf[:].rearrange("p c b -> p (c b)"), pos_f[:]
            )

            # ---- box bounds: xm1 = (cx - w/2)*64 - 1, etc. (f32) ----
            cx = priors_sb[:, :, 0]
            cy = priors_sb[:, :, 1]
            bw = priors_sb[:, :, 2]
            bh = priors_sb[:, :, 3]
            hw_half = small.tile([128, NCHUNK], F32)
            hh_half = small.tile([128, NCHUNK], F32)
            nc.vector.tensor_single_scalar(hw_half[:], bw, 0.5, Op.mult)
            nc.vector.tensor_single_scalar(hh_half[:], bh, 0.5, Op.mult)
            xm1 = small.tile([128, NCHUNK], F32)
            xx1 = small.tile([128, NCHUNK], F32)
            ym1 = small.tile([128, NCHUNK], F32)
            yy1 = small.tile([128, NCHUNK], F32)
            nc.vector.tensor_tensor(xm1[:], cx, hw_half[:], Op.subtract)
            nc.vector.tensor_scalar(xm1[:], xm1[:], 64.0, -1.0, Op.mult, Op.add)
            nc.vector.tensor_tensor(xx1[:], cx, hw_half[:], Op.add)
            nc.vector.tensor_scalar(xx1[:], xx1[:], 64.0, -1.0, Op.mult, Op.add)
            nc.vector.tensor_tensor(ym1[:], cy, hh_half[:], Op.subtract)
            nc.vector.tensor_scalar(ym1[:], ym1[:], 64.0, -1.0, Op.mult, Op.add)
            nc.vector.tensor_tensor(yy1[:], cy, hh_half[:], Op.add)
            nc.vector.tensor_scalar(yy1[:], yy1[:], 64.0, -1.0, Op.mult, Op.add)

            def bcast(bnd):
                return bnd[:].rearrange("p (c o) -> p c o", o=1).broadcast_to(
                    [128, NCHUNK, 64]
                )

            # ---- stream DMA emission: one sync-queue, s,t interleaved per
            # unit so each pair completes together. Last unit split into
            # 1 MB halves for a shorter tail. ----
            unit_tiles = []
            for u in range(NU):
                b, cc = divmod(u, CC)
                src_s = s_flat[b, cc * 128 : (cc + 1) * 128, :]
                src_t = t_flat[b, cc * 128 : (cc + 1) * 128, :]
                if u < NU - 1:
                    s_t = pool_s.tile([128, HW], F32)
                    nc.sync.dma_start(s_t[:], src_s)
                    t_t = pool_t.tile([128, HW], F32)
                    nc.sync.dma_start(t_t[:], src_t)
                    unit_tiles.append(((s_t, t_t), (s_t, t_t)))
                else:
                    HWQ = HW // 4
                    quarters = []
                    for qh in range(4):
                        s_h = pool_s.tile([128, HWQ], F32)
                        nc.sync.dma_start(s_h[:], src_s[:, qh * HWQ : (qh + 1) * HWQ])
                        t_h = pool_t.tile([128, HWQ], F32)
                        nc.sync.dma_start(t_h[:], src_t[:, qh * HWQ : (qh + 1) * HWQ])
                        quarters.append((s_h, t_h))
                    unit_tiles.append(quarters)

            # ---- batched coverage inputs ----
            iota_rep = rastp.tile([128, NCHUNK, 64], BF16)
            nc.gpsimd.iota(
                iota_rep[:],
                pattern=[[0, NCHUNK], [1, 64]],
                base=0,
                channel_multiplier=0,
                allow_small_or_imprecise_dtypes=True,
            )
            covx_all = rastp.tile([128, NCHUNK, 64], BF16)
            covy_all = rastp.tile([128, NCHUNK, 64], BF16)
            tmpB = rastp.tile([128, NCHUNK, 64], BF16)
            # GpSimd computes the three compare planes it can own outright;
            # DVE does A_y and the two combines (interleaved with subtracts
            # below via emission order).
            nc.vector.tensor_tensor(covy_all[:], iota_rep[:], bcast(ym1), Op.is_gt)
            nc.vector.tensor_tensor(covx_all[:], iota_rep[:], bcast(xm1), Op.is_gt)
            nc.vector.tensor_tensor(tmpB[:], iota_rep[:], bcast(xx1), Op.is_le)

            # ---- raster state ----
            hit = ps_hit.tile([64, BPC * 64], F32)
            mask_f = small.tile([64, BPC * 64], F32)
            gmat = small.tile([128, 3 * BPC], F32)
            nc.vector.memset(gmat[:], 0.0)
            cs2_all = small.tile([64, BPC, CC, 64], BF16)

            def emit_xb4(g):
                # (cov_x * pos) for 8 chunks in one DVE op
                xb4 = xb4p.tile([128, GCH, BPC, 64], BF16, tag="xb4")
                covx_v = (
                    covx_all[:, g * GCH : (g + 1) * GCH, :]
                    .rearrange("p c (o w) -> p c o w", o=1)
                    .broadcast_to([128, GCH, BPC, 64])
                )
                pos_v = (
                    pos_bf[:, g * GCH : (g + 1) * GCH, :]
                    .rearrange("p c (b o) -> p c b o", o=1)
                    .broadcast_to([128, GCH, BPC, 64])
                )
                nc.vector.tensor_tensor(xb4[:], covx_v, pos_v, Op.mult)
                return xb4

            def emit_raster_mms(g, xb4):
                for k in range(GCH):
                    c = g * GCH + k
                    nc.tensor.matmul(
                        hit[:],
                        covy_all[:, c, :],
                        xb4[:, k].rearrange("p b w -> p (b w)"),
                        start=(c == 0),
                        stop=(c == NCHUNK - 1),
                    )

            # ---- per-unit stream compute (per [128, 2048] half) ----
            cs_rows = [None] * NU

            GP_SUB_UNITS = (0, 2, 3, 4)

            def emit_half_compute(u, hh, cs_row):
                s_x, t_x = unit_tiles[u][hh]
                sl = slice(hh * HWH, (hh + 1) * HWH)
                d_ap = s_x[:, sl]
                if u not in GP_SUB_UNITS:
                    nc.vector.tensor_tensor(d_ap, d_ap, t_x[:, sl], Op.subtract)
                d2 = d2hp.tile([128, HWH], F32R)
                nc.scalar.activation(d2[:], d_ap, AF.Square)
                cs_ps = ps_cs.tile([1, 2048], F32)
                for q in range(4):
                    nc.tensor.matmul(
                        cs_ps[0:1, q * 512 : (q + 1) * 512],
                        ones_r[:],
                        d2[:, q * 512 : (q + 1) * 512],
                        start=True,
                        stop=True,
                    )
                nc.scalar.copy(cs_row[0:1, hh * 2048 : (hh + 1) * 2048], cs_ps[:])

            def emit_quarter_compute(u, qh, cs_row):
                # last unit drains at 512 KB granularity so the pipeline
                # chases the stream instead of starting after it
                HWQ = HW // 4
                s_x, t_x = unit_tiles[u][qh]
                nc.vector.tensor_tensor(s_x[:], s_x[:], t_x[:], Op.subtract)
                d2 = d2hp.tile([128, HWQ], F32R, tag="d2q")
                nc.scalar.activation(d2[:], s_x[:], AF.Square)
                cs_ps = ps_cs.tile([1, 1024], F32, tag="csq")
                for q in range(2):
                    nc.tensor.matmul(
                        cs_ps[0:1, q * 512 : (q + 1) * 512],
                        ones_r[:],
                        d2[:, q * 512 : (q + 1) * 512],
                        start=True,
                        stop=True,
                    )
                nc.scalar.copy(cs_row[0:1, qh * HWQ : (qh + 1) * HWQ], cs_ps[:])

            def emit_unit_compute(u):
                cs_row = csp.tile([1, HW], BF16, tag="cs_row")
                cs_rows[u] = cs_row
                if u < NU - 1:
                    if u in GP_SUB_UNITS:
                        s_t, t_t = unit_tiles[u][0]
                        nc.gpsimd.tensor_tensor(s_t[:], s_t[:], t_t[:], Op.subtract)
                    for hh in range(2):
                        emit_half_compute(u, hh, cs_row)
                else:
                    for qh in range(4):
                        emit_quarter_compute(u, qh, cs_row)

            def emit_cs_reshape(u):
                # [1, 4096] row -> [64, 64] slice of cs2_all in the mask's
                # own [h, w] layout. SWDGE queue: carries no stream traffic.
                b, cc = divmod(u, CC)
                nc.gpsimd.dma_start(
                    cs2_all[:, b, cc, :],
                    cs_rows[u][:].rearrange("o (p j) -> o p j", j=64),
                )

            # ---- interleaved emission: subtract halves must interleave
            # with the raster work in DVE program order ----
            xb4_cur = None
            for u in range(NU):
                if u == NU - 1:
                    # finish the raster + mask BEFORE the last unit's DVE
                    # subtract so they stay off the serial tail
                    emit_raster_mms(NU - 2, xb4_cur)
                    xb4_cur = emit_xb4(NU - 1)
                    emit_raster_mms(NU - 1, xb4_cur)
                    nc.vector.tensor_single_scalar(mask_f[:], hit[:], 0.0, Op.is_gt)
                    for b in range(BPC):
                        nc.vector.tensor_reduce(
                            gmat[0:64, 2 * BPC + b : 2 * BPC + b + 1],
                            mask_f[:, b * 64 : (b + 1) * 64],
                            mybir.AxisListType.X,
                            Op.add,
                        )
                emit_unit_compute(u)
                if u == 0:
                    # combine cov_x (after A_x/B_x land)
                    nc.vector.tensor_tensor(covx_all[:], covx_all[:], tmpB[:], Op.mult)
                    # B_y reuses tmpB (WAR-serialized after the combine)
                    nc.vector.tensor_tensor(tmpB[:], iota_rep[:], bcast(yy1), Op.is_le)
                    xb4_cur = emit_xb4(0)
                elif u == 1:
                    emit_raster_mms(0, xb4_cur)
                    xb4_cur = emit_xb4(1)
                    # combine cov_y
                    nc.vector.tensor_tensor(covy_all[:], covy_all[:], tmpB[:], Op.mult)
                elif u < NU - 1:
                    emit_raster_mms(u - 1, xb4_cur)
                    xb4_cur = emit_xb4(u)
                if u >= 2:
                    emit_cs_reshape(u - 2)

            # ---- batched half-dots: gmat[0:64, u] = per-partition sums ----
            def emit_dot_batch(b0, b1, col0):
                nb = b1 - b0
                scr = small.tile([64, nb * CC * 64], F32, tag=f"scr{col0}")
                mask_v = (
                    mask_f[:, b0 * 64 : b1 * 64]
                    .rearrange("h (b o w) -> h b o w", o=1, w=64)
                    .broadcast_to([64, nb, CC, 64])
                )
                nc.vector.tensor_tensor(
                    scr[:].rearrange("h (b c w) -> h b c w", b=nb, c=CC),
                    cs2_all[:, b0:b1, :, :],
                    mask_v,
                    Op.mult,
                )
                for k in range(nb * CC):
                    nc.vector.tensor_reduce(
                        gmat[0:64, col0 + k : col0 + k + 1],
                        scr[:, k * 64 : (k + 1) * 64],
                        mybir.AxisListType.X,
                        Op.add,
                    )

            emit_dot_batch(0, 3, 0)   # units 0..5 (samples 0-2)
            emit_cs_reshape(NU - 2)
            emit_cs_reshape(NU - 1)
            emit_dot_batch(3, 4, 6)   # units 6..7 (sample 3, the tail)

            # ---- final partition reduce: out[0, i] = sum_p gmat[p, i] ----
            gout = ps_out.tile([1, 3 * BPC], F32)
            nc.tensor.matmul(gout[:], ones_f[:], gmat[:], start=True, stop=True)
            out_sb = small.tile([1, 3 * BPC], F32)
            nc.scalar.copy(out_sb[:], gout[:])
            nc.sync.dma_start(out[:], out_sb[:])

    nc.compile()
    return nc


_NC_CACHE = {}


def _get_nc():
    if "nc" not in _NC_CACHE:
        _NC_CACHE["nc"] = build_nc()
    return _NC_CACHE["nc"]


def make_in_maps(map_t2, map_s2, priors, mimic_label):
    in_maps = []
    for ci in range(N_CORES):
        sl = slice(ci * BPC, (ci + 1) * BPC)
        lab = np.asarray(mimic_label[sl]).astype(np.int32)  # [BPC, P]
        # labels_r[l, c*BPC + b] = lab[b, l*64 + c]
        labels_r = np.ascontiguousarray(
            lab.reshape(BPC, 128, NCHUNK).transpose(1, 2, 0).reshape(128, NCHUNK * BPC)
        )
        in_maps.append(
            {
                "s": np.ascontiguousarray(map_s2[sl]).astype(np.float32),
                "t": np.ascontiguousarray(map_t2[sl]).astype(np.float32),
                "priors": np.ascontiguousarray(priors).astype(np.float32),
                "labels_r": labels_r,
            }
        )
    return in_maps


def finish_host(core_outs):
    """core_outs: list of [1, 3*BPC] arrays -> scalar loss (float32)."""
    contribs = np.empty(B, np.float64)
    msums = np.empty(B, np.float64)
    for ci in range(N_CORES):
        o = np.asarray(core_outs[ci], dtype=np.float64)
        for b in range(BPC):
            contribs[ci * BPC + b] = o[0, b * CC] + o[0, b * CC + 1]
            msums[ci * BPC + b] = o[0, 2 * BPC + b]
    loss = 0.0
    for i in range(B):
        if msums[i] == 0.0:
            loss = 0.0
        else:
            loss = loss + contribs[i] / msums[i] / C
    return np.float32(loss / B)


def kernel(map_t2, map_s2, priors, mimic_label):
    from concourse.bass_utils import run_bass_kernel_spmd

    nc = _get_nc()
    in_maps = make_in_maps(map_t2, map_s2, priors, mimic_label)
    res = run_bass_kernel_spmd(nc, in_maps, core_ids=list(range(N_CORES)))
    outs = [res.results[ci]["out"] for ci in range(N_CORES)]
    return finish_host(outs)



# revision 12
# speedup vs baseline: 1.0935x; 1.0935x over previous
"""Trainium2 Bass kernel for nn_MimicLoss (masked-MSE mimic loss), v8.

Data-parallel over batch: 8 NeuronCores x 4 samples. Per core:
  1. rasterize per-sample union-of-positive-boxes masks: priors land
     with prior p = lane*64 + c on partition `lane`, chunk `c`;
     coverage built in 6 batched [128, 4096] bf16 DVE ops, xb4
     (cov_x * pos) batched 16 chunks per DVE op, then one bf16 matmul
     per 128-prior chunk accumulates
       hit[h, (b,w)] += sum_p cov_y[p,h] * (cov_x[p,w] * pos[p,b])
     mask = hit > 0; mask also transposed to per-sample row layout
     mask_rows[b, (h w)] via 4 small SBUF->SBUF DMAs.
  2. stream units u = (pixel-half h, sample b): tile [128, cc, j] f32
     holds s[b, :, h*2048:(h+1)*2048]; s and t interleave on the sync
     HWDGE queue (one queue sustains ~375 GB/s). d = s - t computed in
     place in the s tile: GpSimd takes 4 units' subtracts, DVE the
     rest (both engines stay under the ~94 us DMA floor). ACT squares
     to f32r; PE accumulates per-sample column sums into ONE psum tile
     [4, 2048] per pixel-half using per-sample indicator stationaries
     sel_b [128, 4] (only column b set), so sample b's colsum
     accumulates in psum row b across the phase. No per-unit PSUM
     copies, no PSUM->SBUF staging.
  3. per pixel-half: DVE multiplies the psum rows directly with
     mask_rows in place, one reduce per region -> ccol columns; msum
     via one reduce over mask_rows. Host combines the [4, 4] outputs:
     contrib = cols 0..2, msum = col 3; applies /(msum*C) and the
     empty-mask-resets-loss scan, /B.

Self-contained: shapes hardcoded for map_t2/map_s2 [32,256,64,64] f32,
priors [8192,4] f32, mimic_label [32,8192] int32.
"""
import sys

sys.path.insert(0, "/opt/trn_rl_repo")

import numpy as np

import concourse.bacc as bacc
import concourse.tile as tile
from concourse import mybir
from concourse.alu_op_type import AluOpType as Op

F32 = mybir.dt.float32
F32R = mybir.dt.float32r
I32 = mybir.dt.int32
BF16 = mybir.dt.bfloat16
AF = mybir.ActivationFunctionType

B, C, H, W = 32, 256, 64, 64
P = 8192
N_CORES = 8
BPC = B // N_CORES          # samples per core
HW = H * W                  # 4096
HWH = HW // 2               # 2048 pixels per half
HWQ = HWH // 2              # 1024 pixels per tail part
NCHUNK = 64                 # prior chunks; prior p = lane*64 + c
CC = C // 128               # channel chunks
GCH = 16                    # chunks per xb4 batch group
NGR = NCHUNK // GCH         # 4 raster groups
NU = 2 * BPC                # stream units (pixel-half, sample)
GP_SUB_UNITS = (0, 2, 3, 5)  # units whose subtract runs on GpSimd


def build_nc():
    nc = bacc.Bacc("TRN2", debug=False)

    s = nc.dram_tensor("s", [BPC, C, H, W], F32, kind="ExternalInput")
    t = nc.dram_tensor("t", [BPC, C, H, W], F32, kind="ExternalInput")
    priors = nc.dram_tensor("priors", [P, 4], F32, kind="ExternalInput")
    # labels_r[l, c*BPC + b] = mimic_label[b, l*64 + c] (host-permuted)
    labels_r = nc.dram_tensor("labels_r", [128, NCHUNK * BPC], I32, kind="ExternalInput")
    out = nc.dram_tensor("out", [BPC, 4], F32, kind="ExternalOutput")

    # unit (b, h) tile layout: [p, cc, j] = x[b, cc*128 + p, h*2048 + j]
    s_r = s[:].rearrange("b (cc p) (h r) w -> b h p cc (r w)", cc=CC, h=2)
    t_r = t[:].rearrange("b (cc p) (h r) w -> b h p cc (r w)", cc=CC, h=2)
    # prior p = lane*64 + c -> priors_sb[lane, c, j] (contiguous load)
    priors_ap = priors[:].rearrange("(l c) j -> l c j", l=128)

    with tile.TileContext(nc) as tc:
        with (
            tc.tile_pool(name="const", bufs=1) as constp,
            tc.tile_pool(name="small", bufs=1) as small,
            tc.tile_pool(name="rast", bufs=1) as rastp,
            tc.tile_pool(name="xb4p", bufs=2) as xb4p,
            tc.tile_pool(name="stream_s", bufs=4) as pool_s,
            tc.tile_pool(name="stream_t", bufs=2) as pool_t,
            tc.tile_pool(name="d2p", bufs=2) as d2p,
            tc.tile_pool(name="ps_hit", bufs=1, space="PSUM") as ps_hit,
            tc.tile_pool(name="ps_cs", bufs=1, space="PSUM") as ps_cs,
        ):
            # ---- constants & small inputs ----
            # sel3d[:, b, :] = stationary for sample b (only column b set)
            sel_f = constp.tile([128, BPC, BPC], F32)
            sel_r = constp.tile([128, BPC, BPC], F32R)

            priors_sb = small.tile([128, NCHUNK, 4], F32)
            nc.scalar.dma_start(priors_sb[:], priors_ap)
            labels_sb = small.tile([128, NCHUNK, BPC], I32)
            nc.scalar.dma_start(
                labels_sb[:], labels_r[:].rearrange("l (c b) -> l c b", b=BPC)
            )

            # ---- stream tiles + DMA emission (sync HWDGE queue) ----
            # unit u: h = u // BPC, b = u % BPC. Last unit split into
            # column-halves (parts) for a progressive tail drain.
            def unit_bh(u):
                h, b = divmod(u, BPC)
                return b, h

            unit_parts = [None] * NU  # list of (s_tile, t_tile, col0, ncols)
            for u in range(NU):
                b, h = unit_bh(u)
                s_t = pool_s.tile([128, CC, HWH], F32, tag="s", name=f"s_{u}")
                t_t = pool_t.tile([128, CC, HWH], F32, tag="t", name=f"t_{u}")
                if u < NU - 1:
                    nc.sync.dma_start(s_t[:], s_r[b, h])
                    nc.sync.dma_start(t_t[:], t_r[b, h])
                    unit_parts[u] = [(s_t, t_t, 0, HWH)]
                else:
                    parts = []
                    for ph in range(2):
                        cols = slice(ph * HWQ, (ph + 1) * HWQ)
                        nc.sync.dma_start(s_t[:, :, cols], s_r[b, h][:, :, cols])
                        nc.sync.dma_start(t_t[:, :, cols], t_r[b, h][:, :, cols])
                        parts.append((s_t, t_t, ph * HWQ, HWQ))
                    unit_parts[u] = parts

            # ---- raster tiles ----
            iota_rep = rastp.tile([128, NCHUNK, 64], BF16)
            covx_all = rastp.tile([128, NCHUNK, 64], BF16)
            covy_all = rastp.tile([128, NCHUNK, 64], BF16)
            tmpB = rastp.tile([128, NCHUNK, 64], BF16)
            pos_f = small.tile([128, NCHUNK * BPC], F32)
            pos_bf = small.tile([128, NCHUNK, BPC], BF16)
            hw_half = small.tile([128, NCHUNK], F32)
            hh_half = small.tile([128, NCHUNK], F32)
            xm1 = small.tile([128, NCHUNK], F32)
            xx1 = small.tile([128, NCHUNK], F32)
            ym1 = small.tile([128, NCHUNK], F32)
            yy1 = small.tile([128, NCHUNK], F32)

            hit = ps_hit.tile([64, BPC * 64], F32)
            mask_f = small.tile([64, BPC * 64], F32)
            mask_rows = small.tile([BPC, HW], F32)
            ccol = small.tile([BPC, 4], F32)

            def bcast(bnd):
                return bnd[:].rearrange("p (c o) -> p c o", o=1).broadcast_to(
                    [128, NCHUNK, 64]
                )

            # ---- gpsimd program start: iota ----
            nc.gpsimd.iota(
                iota_rep[:],
                pattern=[[0, NCHUNK], [1, 64]],
                base=0,
                channel_multiplier=0,
                allow_small_or_imprecise_dtypes=True,
            )

            # ---- DVE program start: sel stationaries, then raster prep ----
            nc.vector.memset(sel_f[:], 0.0)
            for b in range(BPC):
                nc.vector.memset(sel_f[:, b, b : b + 1], 1.0)
            nc.vector.tensor_copy(
                sel_r[:].rearrange("p a b -> p (a b)"),
                sel_f[:].rearrange("p a b -> p (a b)"),
            )

            nc.vector.tensor_copy(
                pos_f[:], labels_sb[:].rearrange("p c b -> p (c b)")
            )
            nc.vector.tensor_single_scalar(pos_f[:], pos_f[:], 0.0, Op.is_gt)
            nc.vector.tensor_copy(
                pos_bf[:].rearrange("p c b -> p (c b)"), pos_f[:]
            )
            cx = priors_sb[:, :, 0]
            cy = priors_sb[:, :, 1]
            bw = priors_sb[:, :, 2]
            bh = priors_sb[:, :, 3]
            nc.vector.tensor_single_scalar(hw_half[:], bw, 0.5, Op.mult)
            nc.vector.tensor_single_scalar(hh_half[:], bh, 0.5, Op.mult)
            nc.vector.tensor_tensor(xm1[:], cx, hw_half[:], Op.subtract)
            nc.vector.tensor_scalar(xm1[:], xm1[:], 64.0, -1.0, Op.mult, Op.add)
            nc.vector.tensor_tensor(xx1[:], cx, hw_half[:], Op.add)
            nc.vector.tensor_scalar(xx1[:], xx1[:], 64.0, -1.0, Op.mult, Op.add)
            nc.vector.tensor_tensor(ym1[:], cy, hh_half[:], Op.subtract)
            nc.vector.tensor_scalar(ym1[:], ym1[:], 64.0, -1.0, Op.mult, Op.add)
            nc.vector.tensor_tensor(yy1[:], cy, hh_half[:], Op.add)
            nc.vector.tensor_scalar(yy1[:], yy1[:], 64.0, -1.0, Op.mult, Op.add)

            nc.vector.tensor_tensor(covy_all[:], iota_rep[:], bcast(ym1), Op.is_gt)
            nc.vector.tensor_tensor(covx_all[:], iota_rep[:], bcast(xm1), Op.is_gt)
            nc.vector.tensor_tensor(tmpB[:], iota_rep[:], bcast(xx1), Op.is_le)
            nc.vector.tensor_tensor(covx_all[:], covx_all[:], tmpB[:], Op.mult)
            nc.vector.tensor_tensor(tmpB[:], iota_rep[:], bcast(yy1), Op.is_le)
            nc.vector.tensor_tensor(covy_all[:], covy_all[:], tmpB[:], Op.mult)

            def emit_xb4(g):
                xb4 = xb4p.tile([128, GCH, BPC, 64], BF16, tag="xb4", name="xb4")
                covx_v = (
                    covx_all[:, g * GCH : (g + 1) * GCH, :]
                    .rearrange("p c (o w) -> p c o w", o=1)
                    .broadcast_to([128, GCH, BPC, 64])
                )
                pos_v = (
                    pos_bf[:, g * GCH : (g + 1) * GCH, :]
                    .rearrange("p c (b o) -> p c b o", o=1)
                    .broadcast_to([128, GCH, BPC, 64])
                )
                nc.vector.tensor_tensor(xb4[:], covx_v, pos_v, Op.mult)
                return xb4

            def emit_raster_mms(g, xb4):
                for k in range(GCH):
                    c = g * GCH + k
                    nc.tensor.matmul(
                        hit[:],
                        covy_all[:, c, :],
                        xb4[:, k].rearrange("p b w -> p (b w)"),
                        start=(c == 0),
                        stop=(c == NCHUNK - 1),
                    )

            # ---- per-unit stream compute ----
            cs_ps = [None, None]

            def emit_unit_compute(u):
                b, h = unit_bh(u)
                if b == 0:
                    cs_ps[h] = ps_cs.tile([BPC, HWH], F32, tag="cs", name=f"cs{h}")
                first = b == 0
                last = b == BPC - 1
                for s_t, t_t, col0, ncols in unit_parts[u]:
                    cols = slice(col0, col0 + ncols)
                    if u in GP_SUB_UNITS:
                        nc.gpsimd.tensor_tensor(
                            s_t[:, :, cols], s_t[:, :, cols], t_t[:, :, cols],
                            Op.subtract,
                        )
                    else:
                        nc.vector.tensor_tensor(
                            s_t[:, :, cols], s_t[:, :, cols], t_t[:, :, cols],
                            Op.subtract,
                        )
                    d2 = d2p.tile([128, CC, HWH], F32R, tag="d2", name="d2")
                    nc.scalar.activation(d2[:, :, cols], s_t[:, :, cols], AF.Square)
                    nq = ncols // 512
                    for cc in range(CC):
                        for q in range(nq):
                            c0 = col0 + q * 512
                            nc.tensor.matmul(
                                cs_ps[h][0:BPC, c0 : c0 + 512],
                                sel_r[:, b, :],
                                d2[:, cc, c0 : c0 + 512],
                                start=(first and cc == 0),
                                stop=(last and cc == CC - 1),
                            )

            # ---- mask transpose + per-half psum dot ----
            def emit_mask_rows():
                # 4 SBUF->SBUF DMAs: [64, 64] block -> one [1, 4096] row
                for b in range(BPC):
                    nc.gpsimd.dma_start(
                        mask_rows[b : b + 1, :].rearrange("o (p j) -> o p j", j=64),
                        mask_f[:, b * 64 : (b + 1) * 64],
                    )

            def emit_dot(h, col0, ncols, ccol_idx):
                # in-place: mask_rows *= cs_ps rows, then reduce the region
                cols = slice(col0, col0 + ncols)
                mcols = slice(h * HWH + col0, h * HWH + col0 + ncols)
                nc.vector.tensor_tensor(
                    mask_rows[:, mcols], cs_ps[h][:, cols], mask_rows[:, mcols],
                    Op.mult,
                )
                nc.vector.tensor_reduce(
                    ccol[:, ccol_idx : ccol_idx + 1],
                    mask_rows[:, mcols],
                    mybir.AxisListType.X,
                    Op.add,
                )

            # ---- main emission loop ----
            for u in range(NU):
                emit_unit_compute(u)
                # two raster groups per unit -> raster done by u == 1
                if u < NGR // 2:
                    for g in (2 * u, 2 * u + 1):
                        xb4 = emit_xb4(g)
                        emit_raster_mms(g, xb4)
                if u == 2:
                    # raster closed at u == 1: mask, its row transpose,
                    # msum (all independent of the cs stream)
                    nc.vector.tensor_single_scalar(mask_f[:], hit[:], 0.0, Op.is_gt)
                    emit_mask_rows()
                    nc.vector.tensor_reduce(
                        ccol[:, 3:4], mask_rows[:], mybir.AxisListType.X, Op.add
                    )
                if u == BPC + 1:
                    # h=0 psum closed at u == BPC-1: dot it while h=1 streams
                    emit_dot(0, 0, HWH, 0)

            # h=1 tail: last unit was split into column-halves, so the
            # psum segments close progressively
            emit_dot(1, 0, HWQ, 1)
            emit_dot(1, HWQ, HWQ, 2)
            nc.sync.dma_start(out[:], ccol[:])

    nc.compile()
    return nc


_NC_CACHE = {}


def _get_nc():
    if "nc" not in _NC_CACHE:
        _NC_CACHE["nc"] = build_nc()
    return _NC_CACHE["nc"]


def make_in_maps(map_t2, map_s2, priors, mimic_label):
    in_maps = []
    for ci in range(N_CORES):
        sl = slice(ci * BPC, (ci + 1) * BPC)
        lab = np.asarray(mimic_label[sl]).astype(np.int32)  # [BPC, P]
        # labels_r[l, c*BPC + b] = lab[b, l*64 + c]
        labels_r = np.ascontiguousarray(
            lab.reshape(BPC, 128, NCHUNK).transpose(1, 2, 0).reshape(128, NCHUNK * BPC)
        )
        in_maps.append(
            {
                "s": np.ascontiguousarray(map_s2[sl]).astype(np.float32),
                "t": np.ascontiguousarray(map_t2[sl]).astype(np.float32),
                "priors": np.ascontiguousarray(priors).astype(np.float32),
                "labels_r": labels_r,
            }
        )
    return in_maps


def finish_host(core_outs):
    """core_outs: list of [BPC, 4] arrays -> scalar loss (float32)."""
    contribs = np.empty(B, np.float64)
    msums = np.empty(B, np.float64)
    for ci in range(N_CORES):
        o = np.asarray(core_outs[ci], dtype=np.float64)
        for b in range(BPC):
            contribs[ci * BPC + b] = o[b, 0] + o[b, 1] + o[b, 2]
            msums[ci * BPC + b] = o[b, 3]
    loss = 0.0
    for i in range(B):
        if msums[i] == 0.0:
            loss = 0.0
        else:
            loss = loss + contribs[i] / msums[i] / C
    return np.float32(loss / B)


def kernel(map_t2, map_s2, priors, mimic_label):
    from concourse.bass_utils import run_bass_kernel_spmd

    nc = _get_nc()
    in_maps = make_in_maps(map_t2, map_s2, priors, mimic_label)
    res = run_bass_kernel_spmd(nc, in_maps, core_ids=list(range(N_CORES)))
    outs = [res.results[ci]["out"] for ci in range(N_CORES)]
    return finish_host(outs)
